# revision 3
# baseline (speedup 1.0000x reference)
"""Trainium2 Bass kernel for a linear-attention block (ELU+1 feature map).

Single-launch fused design (v3):
  Phase A1: K/V projections (bf16, channel-major), feature map k'=elu+1,
            kv=k'*(v+bv), block-local exclusive cumsums (immediate-0 initial;
            AP initials cost ~4us/scan) kept in SBUF.
  Offsets:  block totals are accumulated into per-(ci,tb) offset columns and
            the pairwise AllReduce adds the partner core's totals (masked so
            only the h=0 core contributes); offsets are applied inside the
            phase-B elementwise ops, so h=0 cores add exact zeros.
  Phase A2: Q projection + feature map.
  Phase B:  p1=(sk+off)*qf -> denominators (hm matmul), reciprocal,
            replicate (hmT matmul), attn = (skv+off)*qf*rep, Wo projection,
            residual + LayerNorm.

Sharding: 8 cores = (batch b in 0..3) x (L-half h in 0..1); each core owns
2048 contiguous rows of one batch.  ONE SPMD launch.
"""

import sys

if "/opt/trn_rl_repo" not in sys.path:
    sys.path.insert(0, "/opt/trn_rl_repo")

import numpy as np
import ml_dtypes

import concourse.bass as bass
import concourse.mybir as mybir
import concourse.tile as tile
import concourse.bass_utils as bass_utils
import concourse.bass2jax as bass2jax
from concourse.bass_utils import run_bass_kernel_spmd


# --------------------------------------------------------------------------
# Compile fix: the walrus build in this container rejects instructions whose
# sync_info carries more than one on_wait ("Too many sync wait commands").
# Tile attaches multi-wait sync_info; split the extras into standalone
# EventSemaphore instructions (exactly what raw bass emits for wait_ge),
# which this walrus accepts.  Semantics preserved: engines are in-order, so
# waiting before the instruction == waiting on the instruction.
# --------------------------------------------------------------------------
def _split_multi_waits(bir_json):
    import json as _json

    bir = _json.loads(bir_json)
    ctr = 0
    changed = False
    for fn in bir.get("functions", []):
        for blk in fn.get("blocks", []):
            out = []
            for inst in blk.get("instructions", []):
                si = inst.get("sync_info")
                waits = (si or {}).get("on_wait") or []
                if len(waits) > 1:
                    for w in waits[:-1]:
                        ctr += 1
                        out.append({
                            "name": f"EVSx-{ctr}",
                            "opcode": "EventSemaphore",
                            "engine": inst["engine"],
                            "ins": [], "outs": [],
                            "sync_info": {"on_update": [], "on_wait": [w]},
                        })
                    si["on_wait"] = waits[-1:]
                    changed = True
                out.append(inst)
            blk["instructions"] = out
    if not changed:
        return bir_json
    return _json.dumps(bir).encode()


_orig_compile_bir_kernel = bass_utils.compile_bir_kernel


def _compile_bir_kernel_splitwaits(bir_json, tmpdir, neff_name="file.neff"):
    return _orig_compile_bir_kernel(_split_multi_waits(bir_json), tmpdir, neff_name)


if getattr(bass_utils.compile_bir_kernel, "__name__", "") != (
    "_compile_bir_kernel_splitwaits"
):
    bass_utils.compile_bir_kernel = _compile_bir_kernel_splitwaits
    bass2jax.compile_bir_kernel = _compile_bir_kernel_splitwaits

BF16 = ml_dtypes.bfloat16
F32 = np.float32

B, L, DM, H, D = 4, 4096, 1024, 16, 64
NCORES = 8
LH = L // 2          # 2048 rows per core
P = 128              # partitions
NCH = DM // P        # 8 channel chunks of 128
HPC = P // D         # 2 heads per channel chunk
TB = 512             # token block (matmul free dim)
NTB = LH // TB       # 4 token blocks per core
SEG = TB + 1         # per-block segment in the prefix tensors
EPS_ATTN = 1e-9
EPS_LN = 1e-6

_FP = mybir.dt.float32
_BF = mybir.dt.bfloat16
_ALU = mybir.AluOpType
_ACTF = mybir.ActivationFunctionType

# toggles for test harness
TRACE = False
LAST_PROFILE = {}

REPLICA_PAIRS = [[0, 1], [2, 3], [4, 5], [6, 7]]


def build_fused(trivial_gb):
    nc = bass.Bass(name="linattn_fused", num_devices=NCORES)
    qT = nc.dram_tensor("qT", [DM, LH], _BF, kind="ExternalInput")
    kT = nc.dram_tensor("kT", [DM, LH], _BF, kind="ExternalInput")
    vT = nc.dram_tensor("vT", [DM, LH], _BF, kind="ExternalInput")
    wq = nc.dram_tensor("wq", [P, NCH, DM], _BF, kind="ExternalInput")
    wk = nc.dram_tensor("wk", [P, NCH, DM], _BF, kind="ExternalInput")
    wv = nc.dram_tensor("wv", [P, NCH, DM], _BF, kind="ExternalInput")
    bqkv = nc.dram_tensor("bqkv", [P, 3 * NCH], _FP, kind="ExternalInput")
    wo = nc.dram_tensor("wo", [P, NCH, DM], _BF, kind="ExternalInput")
    hm = nc.dram_tensor("hm", [P, NCH, H], _BF, kind="ExternalInput")
    hmT = nc.dram_tensor("hmT", [H, NCH, P], _BF, kind="ExternalInput")
    qrows = nc.dram_tensor("qrows", [LH, DM], _BF, kind="ExternalInput")
    # cmask: col0 = 1.0 iff this core owns the FIRST half (contributes totals),
    #        col1 = 1.0 iff this core owns the SECOND half (uses offsets)
    cmask = nc.dram_tensor("cmask", [P, 2], _FP, kind="ExternalInput")
    if not trivial_gb:
        gb = nc.dram_tensor("gb", [2, DM], _FP, kind="ExternalInput")

    out = nc.dram_tensor("out", [LH, DM], _FP, kind="ExternalOutput")

    x_view = {
        "q": qT.rearrange("(o p) t -> p o t", p=P),
        "k": kT.rearrange("(o p) t -> p o t", p=P),
        "v": vT.rearrange("(o p) t -> p o t", p=P),
    }

    with tile.TileContext(nc) as tc:
        with (
            tc.tile_pool(name="wpool", bufs=1) as wpool,
            tc.tile_pool(name="cpool", bufs=1) as cpool,
            tc.tile_pool(name="skpool", bufs=1) as skpool,
            tc.tile_pool(name="xq", bufs=2) as xq,
            tc.tile_pool(name="pspool", bufs=3, space="PSUM") as pspool,
            tc.tile_pool(name="dram", bufs=2, space="DRAM") as drampool,
        ):
            # K/V weights first: the first matmuls need them
            w_sb = {}
            for name, t in (("k", wk), ("v", wv)):
                w_sb[name] = wpool.tile(
                    [P, NCH, DM], _BF, tag=f"w{name}", name=f"w{name}"
                )
                nc.sync.dma_start(w_sb[name][:], t[:])
            bias_sb = cpool.tile([P, 3 * NCH], _FP, tag="bias")
            nc.sync.dma_start(bias_sb[:], bqkv[:])
            hm_sb = cpool.tile([P, NCH, H], _BF, tag="hm")
            nc.sync.dma_start(hm_sb[:], hm[:])
            hmT_sb = cpool.tile([H, NCH, P], _BF, tag="hmT")
            nc.sync.dma_start(hmT_sb[:], hmT[:])

            # exclusive-prefix tensors, block-local: segment tb occupies
            # [tb*SEG, (tb+1)*SEG); element 0 of a segment is zero, the scan
            # writes [1, TB], the block-exclusive view is [0, TB) and the
            # block total sits at tb*SEG+TB.
            sk_full = skpool.tile([P, NCH, NTB * SEG], _BF, tag="skf", name="skf")
            skv_full = skpool.tile([P, NCH, NTB * SEG], _BF, tag="skvf", name="skvf")
            for tb in range(NTB):
                nc.vector.memset(sk_full[:, :, tb * SEG:tb * SEG + 1], 0.0)
                nc.vector.memset(skv_full[:, :, tb * SEG:tb * SEG + 1], 0.0)

            # ---------------- phase A1: K/V proj + fmap + scans ----------------
            with (
                tc.tile_pool(name="xkv", bufs=2) as xkv,
                tc.tile_pool(name="scanb", bufs=2) as scanb,
                tc.tile_pool(name="fmap", bufs=2) as fmap,
            ):
                for tb in range(NTB):
                    tsl = slice(tb * TB, (tb + 1) * TB)
                    xk_t = xkv.tile([P, NCH, TB], _BF, tag="xk", name="xk")
                    xv_t = xkv.tile([P, NCH, TB], _BF, tag="xv", name="xv")
                    nc.sync.dma_start(xk_t[:], x_view["k"][:, :, tsl])
                    nc.sync.dma_start(xv_t[:], x_view["v"][:, :, tsl])
                    kb_s, kvb_s = {}, {}
                    for ci in range(NCH):
                        csl = slice(ci * P, (ci + 1) * P)
                        ps_k = pspool.tile([P, TB], _FP, tag="pp")
                        for o in range(NCH):
                            nc.tensor.matmul(
                                ps_k, w_sb["k"][:, o, csl], xk_t[:, o, :],
                                start=(o == 0), stop=(o == NCH - 1),
                            )
                        ps_v = pspool.tile([P, TB], _FP, tag="pp")
                        for o in range(NCH):
                            nc.tensor.matmul(
                                ps_v, w_sb["v"][:, o, csl], xv_t[:, o, :],
                                start=(o == 0), stop=(o == NCH - 1),
                            )
                        # k' = min(exp(klin+bk),1) + relu(klin+bk)
                        ek = fmap.tile([P, TB], _BF, tag="ek")
                        rk = fmap.tile([P, TB], _BF, tag="rk")
                        kb_col = bias_sb[:, NCH + ci:NCH + ci + 1]
                        nc.scalar.activation(ek[:], ps_k[:], _ACTF.Exp, bias=kb_col)
                        nc.scalar.activation(rk[:], ps_k[:], _ACTF.Relu, bias=kb_col)
                        mk = fmap.tile([P, TB], _BF, tag="mk")
                        nc.vector.tensor_scalar(mk[:], ek[:], 1.0, None, _ALU.min)
                        kb = scanb.tile([P, TB], _BF, tag=f"kb{ci % 2}")
                        nc.vector.tensor_tensor(kb[:], mk[:], rk[:], _ALU.add)
                        # kv = (vlin + bv) * k' -- emitted before the scans so
                        # the V PSUM bank recycles without waiting on them
                        vb_col = bias_sb[:, 2 * NCH + ci:2 * NCH + ci + 1]
                        kvb = scanb.tile([P, TB], _BF, tag=f"kvb{ci % 2}")
                        nc.vector.scalar_tensor_tensor(
                            kvb[:], ps_v[:], vb_col, kb[:], _ALU.add, _ALU.mult
                        )
                        kb_s[ci], kvb_s[ci] = kb, kvb
                        if ci % 2 == 1:
                            s0 = tb * SEG
                            for cj in (ci - 1, ci):
                                nc.vector.tensor_tensor_scan(
                                    sk_full[:, cj, s0 + 1:s0 + 1 + TB],
                                    kb_s[cj][:], kb_s[cj][:], 0.0,
                                    _ALU.add, _ALU.bypass,
                                )
                                nc.vector.tensor_tensor_scan(
                                    skv_full[:, cj, s0 + 1:s0 + 1 + TB],
                                    kvb_s[cj][:], kvb_s[cj][:], 0.0,
                                    _ALU.add, _ALU.bypass,
                                )

            # ---------------- offsets: block chain + collective ----------------
            # off_blk[:, ci, tb] = sum of this core's block totals before tb
            off_blk1 = cpool.tile([P, NCH, NTB], _FP, tag="ob1")
            off_blk2 = cpool.tile([P, NCH, NTB], _FP, tag="ob2")
            nc.vector.memset(off_blk1[:, :, 0:1], 0.0)
            nc.vector.memset(off_blk2[:, :, 0:1], 0.0)
            contrib = cpool.tile([P, 2 * NCH], _FP, tag="contrib")
            cm_sb = cpool.tile([P, 2], _FP, tag="cm")
            nc.sync.dma_start(cm_sb[:], cmask[:])
            for ci in range(NCH):
                for tb in range(1, NTB):
                    t0 = (tb - 1) * SEG + TB
                    nc.vector.tensor_tensor(
                        off_blk1[:, ci, tb:tb + 1], off_blk1[:, ci, tb - 1:tb],
                        sk_full[:, ci, t0:t0 + 1], _ALU.add,
                    )
                    nc.vector.tensor_tensor(
                        off_blk2[:, ci, tb:tb + 1], off_blk2[:, ci, tb - 1:tb],
                        skv_full[:, ci, t0:t0 + 1], _ALU.add,
                    )
                tl = (NTB - 1) * SEG + TB
                nc.vector.tensor_tensor(
                    contrib[:, ci:ci + 1], off_blk1[:, ci, NTB - 1:NTB],
                    sk_full[:, ci, tl:tl + 1], _ALU.add,
                )
                nc.vector.tensor_tensor(
                    contrib[:, NCH + ci:NCH + ci + 1], off_blk2[:, ci, NTB - 1:NTB],
                    skv_full[:, ci, tl:tl + 1], _ALU.add,
                )
            nc.vector.tensor_scalar(
                contrib[:], contrib[:], cm_sb[:, 0:1], None, _ALU.mult
            )
            cc_in = drampool.tile([P, 2 * NCH], _FP)
            cc_out = drampool.tile([P, 2 * NCH], _FP)
            nc.gpsimd.dma_start(cc_in[:], contrib[:])
            nc.gpsimd.collective_compute(
                "AllReduce",
                _ALU.add,
                replica_groups=REPLICA_PAIRS,
                ins=[cc_in.opt()],
                outs=[cc_out.opt()],
            )
            off_raw = cpool.tile([P, 2 * NCH], _FP, tag="offr")
            nc.gpsimd.dma_start(off_raw[:], cc_out[:])
            # comb[:, ci, tb] = off_blk + (partner total iff second-half core)
            comb1 = cpool.tile([P, NCH, NTB], _FP, tag="comb1")
            comb2 = cpool.tile([P, NCH, NTB], _FP, tag="comb2")
            for ci in range(NCH):
                for tb in range(NTB):
                    nc.vector.scalar_tensor_tensor(
                        comb1[:, ci, tb:tb + 1], off_raw[:, ci:ci + 1],
                        cm_sb[:, 1:2], off_blk1[:, ci, tb:tb + 1],
                        _ALU.mult, _ALU.add,
                    )
                    nc.vector.scalar_tensor_tensor(
                        comb2[:, ci, tb:tb + 1], off_raw[:, NCH + ci:NCH + ci + 1],
                        cm_sb[:, 1:2], off_blk2[:, ci, tb:tb + 1],
                        _ALU.mult, _ALU.add,
                    )
            # hmoff1[:, ci, tb*H:(tb+1)*H] = hm * comb1 column: the denominator
            # offset correction enters via extra matmuls so the local dn
            # accumulation does not wait for the collective
            hmoff1 = cpool.tile([P, NCH, NTB * H], _BF, tag="hmoff1")
            for ci in range(NCH):
                for tb in range(NTB):
                    nc.vector.tensor_scalar(
                        hmoff1[:, ci, tb * H:(tb + 1) * H], hm_sb[:, ci, :],
                        comb1[:, ci, tb:tb + 1], None, _ALU.mult,
                    )

            # ---------------- phase A2 + B, per token block ----------------
            with (
                tc.tile_pool(name="fmap2", bufs=2) as fmap2,
                tc.tile_pool(name="qfp", bufs=2) as qfp,
                tc.tile_pool(name="btmp", bufs=2) as btmp,
                tc.tile_pool(name="att", bufs=2) as att,
                tc.tile_pool(name="apool", bufs=1) as apool,
                tc.tile_pool(name="qrp", bufs=2) as qrp,
                tc.tile_pool(name="lnp", bufs=1) as lnp,
                tc.tile_pool(name="psdn", bufs=1, space="PSUM") as psdn,
                tc.tile_pool(name="psrep", bufs=2, space="PSUM") as psrep,
                tc.tile_pool(name="psao", bufs=2, space="PSUM") as psao,
            ):
                wq_sb = wpool.tile([P, NCH, DM], _BF, tag="wq", name="wq")
                nc.sync.dma_start(wq_sb[:], wq[:])
                wo_sb = wpool.tile([P, NCH, DM], _BF, tag="wo", name="wo")
                nc.sync.dma_start(wo_sb[:], wo[:])
                eps_sb = cpool.tile([P, 1], _FP, tag="eps")
                nc.vector.memset(eps_sb[:], EPS_LN)
                if not trivial_gb:
                    gamma_rep = cpool.tile([P, DM], _FP, tag="gamma")
                    nc.sync.dma_start(gamma_rep[:], gb[0:1, :].to_broadcast([P, DM]))
                    beta_rep = cpool.tile([P, DM], _FP, tag="beta")
                    nc.sync.dma_start(beta_rep[:], gb[1:2, :].to_broadcast([P, DM]))

                for tb in range(NTB):
                    tsl = slice(tb * TB, (tb + 1) * TB)
                    bsl = slice(tb * SEG, tb * SEG + TB)  # block-exclusive view
                    xq_t = xq.tile([P, NCH, TB], _BF, tag="xq", name="xq")
                    nc.sync.dma_start(xq_t[:], x_view["q"][:, :, tsl])
                    qf_t = qfp.tile([P, NCH, TB], _BF, tag="qf", name="qf")
                    for ci in range(NCH):
                        csl = slice(ci * P, (ci + 1) * P)
                        ps_q = pspool.tile([P, TB], _FP, tag="pp")
                        for o in range(NCH):
                            nc.tensor.matmul(
                                ps_q, wq_sb[:, o, csl], xq_t[:, o, :],
                                start=(o == 0), stop=(o == NCH - 1),
                            )
                        eq = fmap2.tile([P, TB], _BF, tag="eq")
                        rq = fmap2.tile([P, TB], _BF, tag="rq")
                        qb_col = bias_sb[:, ci:ci + 1]
                        nc.scalar.activation(eq[:], ps_q[:], _ACTF.Exp, bias=qb_col)
                        nc.scalar.activation(rq[:], ps_q[:], _ACTF.Relu, bias=qb_col)
                        mq = fmap2.tile([P, TB], _BF, tag="mq")
                        nc.vector.tensor_scalar(mq[:], eq[:], 1.0, None, _ALU.min)
                        nc.vector.tensor_tensor(
                            qf_t[:, ci, :], mq[:], rq[:], _ALU.add
                        )

                    # ---------------- phase B for this tb ----------------
                    dn = psdn.tile([H, TB], _FP, tag="dn")
                    for ci in range(NCH):
                        p1 = btmp.tile([P, TB], _BF, tag="p1")
                        nc.vector.tensor_tensor(
                            p1[:], sk_full[:, ci, bsl], qf_t[:, ci, :], _ALU.mult
                        )
                        nc.tensor.matmul(
                            dn[:], hm_sb[:, ci], p1[:],
                            start=(ci == 0), stop=False,
                        )
                    for ci in range(NCH):
                        nc.tensor.matmul(
                            dn[:], hmoff1[:, ci, tb * H:(tb + 1) * H],
                            qf_t[:, ci, :],
                            start=False, stop=(ci == NCH - 1),
                        )
                    dn_sb = att.tile([H, TB], _FP, tag="dnsb")
                    nc.vector.tensor_scalar(
                        dn_sb[:], dn[:], EPS_ATTN, None, _ALU.add
                    )
                    rc = att.tile([H, TB], _BF, tag="rc")
                    with nc.allow_low_precision(reason="bf16 recip feeds bf16 matmul"):
                        nc.vector.reciprocal(rc[:], dn_sb[:])

                    a_tiles = []
                    for ci in range(NCH):
                        rep = psrep.tile([P, TB], _FP, tag="rep")
                        nc.tensor.matmul(
                            rep[:], hmT_sb[:, ci], rc[:], start=True, stop=True
                        )
                        rep_sb = btmp.tile([P, TB], _BF, tag="repsb")
                        nc.scalar.activation(
                            rep_sb[:], rep[:], _ACTF.Identity, bias=0.0
                        )
                        qr = btmp.tile([P, TB], _BF, tag="qr")
                        nc.vector.tensor_tensor(
                            qr[:], qf_t[:, ci, :], rep_sb[:], _ALU.mult
                        )
                        svo = btmp.tile([P, TB], _BF, tag="svo")
                        nc.vector.tensor_scalar(
                            svo[:], skv_full[:, ci, bsl],
                            comb2[:, ci, tb:tb + 1], None, _ALU.add,
                        )
                        a_t = apool.tile([P, TB], _BF, tag=f"a{ci}", name=f"a{ci}")
                        nc.vector.tensor_tensor(a_t[:], svo[:], qr[:], _ALU.mult)
                        a_tiles.append(a_t)

                    # Wo projection + residual + LayerNorm, per 128-row subtile
                    for s4 in range(TB // P):
                        row0 = tb * TB + s4 * P
                        ssl = slice(s4 * P, (s4 + 1) * P)
                        qrow = qrp.tile([P, DM], _BF, tag="qrow")
                        nc.sync.dma_start(qrow[:], qrows[row0:row0 + P, :])
                        x_sb = lnp.tile([P, DM], _FP, tag="x")
                        xs = lnp.tile([P, 2], _FP, tag="xs")
                        for mb in range(DM // TB):
                            msl = slice(mb * TB, (mb + 1) * TB)
                            ao = psao.tile([P, TB], _FP, tag="ao")
                            for ci in range(NCH):
                                nc.tensor.matmul(
                                    ao[:], a_tiles[ci][:, ssl], wo_sb[:, ci, msl],
                                    start=(ci == 0), stop=(ci == NCH - 1),
                                )
                            nc.vector.scalar_tensor_tensor(
                                x_sb[:, msl], ao[:], 0.0, qrow[:, msl],
                                _ALU.add, _ALU.add, accum_out=xs[:, mb:mb + 1],
                            )
                        # LayerNorm stats: DVE supplies sum(x^2)
                        xsq = lnp.tile([P, DM], _BF, tag="xsq")
                        sq = lnp.tile([P, 1], _FP, tag="sq")
                        nc.vector.scalar_tensor_tensor(
                            xsq[:], x_sb[:], 0.0, x_sb[:], _ALU.add, _ALU.mult,
                            accum_out=sq[:, 0:1],
                        )
                        mv = lnp.tile([P, 2], _FP, tag="mv")
                        nc.vector.tensor_tensor(
                            mv[:, 0:1], xs[:, 0:1], xs[:, 1:2], _ALU.add
                        )
                        nc.vector.tensor_scalar_mul(mv[:, 0:1], mv[:, 0:1], 1.0 / DM)
                        nc.vector.tensor_scalar_mul(mv[:, 1:2], sq[:, 0:1], 1.0 / DM)
                        var = lnp.tile([P, 1], _FP, tag="var")
                        nc.vector.scalar_tensor_tensor(
                            var[:], mv[:, 0:1], -1.0, mv[:, 0:1], _ALU.mult, _ALU.mult
                        )
                        nc.vector.tensor_tensor(var[:], var[:], mv[:, 1:2], _ALU.add)
                        rstd = lnp.tile([P, 1], _FP, tag="rstd")
                        nc.scalar.activation(
                            rstd[:], var[:, 0:1], _ACTF.Sqrt, bias=eps_sb[:, 0:1]
                        )
                        nc.vector.reciprocal(rstd[:], rstd[:])
                        y = lnp.tile([P, DM], _FP, tag="y")
                        if trivial_gb:
                            nmr = lnp.tile([P, 1], _FP, tag="nmr")
                            nc.vector.scalar_tensor_tensor(
                                nmr[:], mv[:, 0:1], -1.0, rstd[:], _ALU.mult, _ALU.mult
                            )
                            nc.scalar.activation(
                                y[:], x_sb[:], _ACTF.Identity,
                                bias=nmr[:, 0:1], scale=rstd[:, 0:1],
                            )
                        else:
                            nc.vector.tensor_scalar(
                                y[:], x_sb[:], mv[:, 0:1], rstd[:],
                                _ALU.subtract, _ALU.mult,
                            )
                            nc.gpsimd.tensor_tensor(y[:], y[:], gamma_rep[:], _ALU.mult)
                            nc.gpsimd.tensor_tensor(y[:], y[:], beta_rep[:], _ALU.add)
                        nc.sync.dma_start(out[row0:row0 + P, :], y[:])
    return nc


# --------------------------------------------------------------------------
# Host orchestration
# --------------------------------------------------------------------------
_cache = {}


def _consts():
    if "hm" in _cache:
        return
    hm = np.zeros((P, NCH, H), BF16)
    hmT = np.zeros((H, NCH, P), BF16)
    for o in range(NCH):
        for p in range(P):
            j = o * HPC + p // D
            hm[p, o, j] = 1.0
            hmT[j, o, p] = 1.0
    _cache["hm"] = hm
    _cache["hmT"] = hmT


def _w_chunks(w):
    # (DM, DM) -> (P, NCH, DM): [p, o, c] = w[o*P + p, c]
    return np.ascontiguousarray(
        w.astype(BF16).reshape(NCH, P, DM).transpose(1, 0, 2)
    )


def _col_chunks(v):
    # (DM,) -> (P, NCH): [p, o] = v[o*P + p]
    return np.ascontiguousarray(v.astype(F32).reshape(NCH, P).T)


def kernel(**inputs):
    query = np.ascontiguousarray(np.asarray(inputs["query"], F32))
    key_in = np.asarray(inputs.get("key_in", inputs.get("key")), F32)
    value = np.asarray(inputs["value"], F32)
    Wq, Wk, Wv, Wo = (np.asarray(inputs[k], F32) for k in ("Wq", "Wk", "Wv", "Wo"))
    bq, bk, bv, bo = (np.asarray(inputs[k], F32) for k in ("bq", "bk", "bv", "bo"))
    gamma = np.asarray(inputs["gamma"], F32)
    beta = np.asarray(inputs["beta"], F32)
    trivial_gb = bool((gamma == 1.0).all() and (beta == 0.0).all())

    _consts()
    if ("fused", trivial_gb) not in _cache:
        _cache[("fused", trivial_gb)] = build_fused(trivial_gb)
    nc = _cache[("fused", trivial_gb)]

    wq_c, wk_c, wv_c, wo_c = map(_w_chunks, (Wq, Wk, Wv, Wo))
    bqkv = np.ascontiguousarray(
        np.concatenate([_col_chunks(bq), _col_chunks(bk), _col_chunks(bv)], axis=1)
    )
    gb = np.ascontiguousarray(np.stack([gamma, beta]).astype(F32))

    cmask0 = np.zeros((P, 2), F32)
    cmask0[:, 0] = 1.0
    cmask1 = np.zeros((P, 2), F32)
    cmask1[:, 1] = 1.0

    core_ids = list(range(NCORES))
    in_maps = []
    for c in core_ids:
        b, h = c // 2, c % 2
        rows = slice(h * LH, (h + 1) * LH)
        m = {
            "qT": np.ascontiguousarray(query[b, rows, :].astype(BF16).T),
            "kT": np.ascontiguousarray(key_in[b, rows, :].astype(BF16).T),
            "vT": np.ascontiguousarray(value[b, rows, :].astype(BF16).T),
            "wq": wq_c, "wk": wk_c, "wv": wv_c, "bqkv": bqkv,
            "wo": wo_c, "hm": _cache["hm"], "hmT": _cache["hmT"],
            "qrows": np.ascontiguousarray(
                (query[b, rows, :] + bo).astype(BF16)
            ),
            "cmask": cmask0 if h == 0 else cmask1,
        }
        if not trivial_gb:
            m["gb"] = gb
        in_maps.append(m)

    r = run_bass_kernel_spmd(nc, in_maps, core_ids, trace=TRACE)
    if TRACE:
        LAST_PROFILE["l1_ns"] = r.exec_time_ns
        LAST_PROFILE["l1_json"] = r.profile_json
        LAST_PROFILE["l2_ns"] = 0

    out = np.empty((B, L, DM), F32)
    for c in core_ids:
        b, h = c // 2, c % 2
        out[b, h * LH:(h + 1) * LH, :] = np.asarray(r.results[c]["out"], F32)
    return out
